# revision 1
# baseline (speedup 1.0000x reference)
"""CrossBlock (cross-attention transformer block) Trainium2 Bass kernel.

Problem: B=8, N=M=1024, C=512, H=8 heads (d=64), HID=2048, fp32.
Sharding: pure data-parallel — batch b -> NeuronCore b. No collectives.

Host-side prep (O(C^2), negligible): LN affine gains/biases folded into
the K/V/FFN1 weights:  Wk' = diag(g1) Wk, bk' = ln1_b @ Wk + bk, etc.

Per-core dataflow (natural layout = [n,c] tiles [128, nchunk, 512];
transposed layout = [c, n] tiles [128, cchunk, 1024]):
  LN1(x) (+affine g1,b1) + pos -> xp ; LN1(t) -> t_hat (affine folded)
  PE-transpose xp, t_hat -> xpT, ttT
  qT = Wq^T @ xpT (+bq) ; kT = Wk'^T @ ttT (+bk') ; v = ttT^T @ Wv' (+bv')
  per head h: sT[m,n] = kT_h^T @ qT_h ; E = exp(sT/8) (ScalarE, from PSUM)
              oT_aug = [v_h | 1]^T @ E   (row 64 = softmax denominator)
              oT_h = oT_aug[0:64] * bcast(1/denominator)
  att = oT^T @ Wo ; x2 = (x + bo) + att          (in place over x)
  LN2(x2) -> PE-transpose -> x2T                 (affine folded into W1')
  h = max(z, 0.01 z), z = W1'^T @ x2T + b1' ; out = (x2 + b2) + h^T W2
All matmuls run as float32r (TF32-like, 1 cyc/row on the PE).
"""

import numpy as np

import concourse.bass as bass
import concourse.mybir as mybir
import concourse.tile as tile
from concourse import bacc
from concourse.masks import make_identity

B, N, M, C, H, HID = 8, 1024, 1024, 512, 8, 2048
D = C // H  # 64
P = 128
NCH = N // P    # 8 n-chunks
MCH = M // P    # 8 m-chunks
CCH = C // P    # 4 c-chunks
HCH = HID // P  # 16 hid-chunks
NT = 512        # matmul moving-dim tile (one fp32 PSUM bank)
EPS = 1e-5
F32 = mybir.dt.float32
F32R = mybir.dt.float32r

AF = mybir.ActivationFunctionType
ALU = mybir.AluOpType


def r(ap):
    return ap.bitcast(F32R)


def layer_norm_std(nc, stats, xt, xhat, nchunks, eps_t):
    """xhat[:, ch, :] = (xt[:, ch, :] - mean) * rsqrt(var + eps), per row."""
    for ch in range(nchunks):
        st = stats.tile([P, 6], F32, tag="st")
        mv = stats.tile([P, 2], F32, tag="mv")
        nc.vector.bn_stats(out=st[:], in_=xt[:, ch, :])
        nc.vector.bn_aggr(out=mv[:], in_=st[:])
        sd = stats.tile([P, 1], F32, tag="sd")
        nc.scalar.activation(sd[:], mv[:, 1:2], AF.Sqrt, bias=eps_t[:],
                             scale=1.0)
        rv = stats.tile([P, 1], F32, tag="rv")
        nc.vector.reciprocal(rv[:], sd[:])
        nc.vector.tensor_scalar(
            out=xhat[:, ch, :], in0=xt[:, ch, :],
            scalar1=mv[:, 0:1], scalar2=rv[:],
            op0=ALU.subtract, op1=ALU.mult,
        )


def build_nc():
    nc = bacc.Bacc("TRN2", target_bir_lowering=False, debug=False)

    x_d = nc.dram_tensor("x", [N, C], F32, kind="ExternalInput")
    t_d = nc.dram_tensor("t", [M, C], F32, kind="ExternalInput")
    pos_d = nc.dram_tensor("pos", [N, C], F32, kind="ExternalInput")
    ln1_g = nc.dram_tensor("ln1_g", [C], F32, kind="ExternalInput")
    ln1_b = nc.dram_tensor("ln1_b", [C], F32, kind="ExternalInput")
    Wq_d = nc.dram_tensor("Wq", [C, C], F32R, kind="ExternalInput")
    bq_d = nc.dram_tensor("bq", [C], F32, kind="ExternalInput")
    Wk_d = nc.dram_tensor("Wk", [C, C], F32R, kind="ExternalInput")
    bk_d = nc.dram_tensor("bk", [C], F32, kind="ExternalInput")
    Wv_d = nc.dram_tensor("Wv", [C, C], F32R, kind="ExternalInput")
    bv_d = nc.dram_tensor("bv", [C], F32, kind="ExternalInput")
    Wo_d = nc.dram_tensor("Wo", [C, C], F32R, kind="ExternalInput")
    bo_d = nc.dram_tensor("bo", [C], F32, kind="ExternalInput")
    W1_d = nc.dram_tensor("W1", [C, HID], F32R, kind="ExternalInput")
    b1_d = nc.dram_tensor("b1", [HID], F32, kind="ExternalInput")
    W2_d = nc.dram_tensor("W2", [HID, C], F32R, kind="ExternalInput")
    b2_d = nc.dram_tensor("b2", [C], F32, kind="ExternalInput")
    vones_d = nc.dram_tensor("vones", [H], F32R, kind="ExternalInput")
    out_d = nc.dram_tensor("out", [N, C], F32, kind="ExternalOutput")

    with tile.TileContext(nc) as tc:
        def act_copy(dst, src):
            nc.scalar.activation(dst, src, AF.Copy)

        def transpose_to_T(psum_pool, src, dstT, ident, nchunks, cchunks):
            """src [128, nchunks, cchunks*128] -> dstT [128, cchunks, nchunks*128]."""
            for cc in range(cchunks):
                for j4 in range(nchunks // 4):
                    pt = psum_pool.tile([P, 4, P], F32, tag="ptr")
                    for k in range(4):
                        ch = j4 * 4 + k
                        nc.tensor.transpose(
                            pt[:, k, :], src[:, ch, cc * P:(cc + 1) * P],
                            ident[:])
                    act_copy(r(dstT[:, cc, j4 * 512:(j4 + 1) * 512]), pt[:, :, :])

        # ------- pool creation order defines the (strict LIFO) stack -------
        singles = tc.alloc_tile_pool(name="singles", bufs=1)
        wqkv = tc.alloc_tile_pool(name="wqkv", bufs=1)
        wffn1 = tc.alloc_tile_pool(name="wffn1", bufs=1)
        iop = tc.alloc_tile_pool(name="io", bufs=1)
        stats = tc.alloc_tile_pool(name="stats", bufs=8)

        ident = singles.tile([P, P], F32)
        make_identity(nc, ident[:])

        eps_t = singles.tile([P, 1], F32)
        nc.vector.memset(eps_t[:], EPS)

        def load_cols(dram, kos):
            # [C] -> [128, kos] per-partition columns
            t_ = singles.tile([P, kos], F32, tag=f"cols_{dram.name}",
                              name=f"cols_{dram.name}")
            nc.gpsimd.dma_start(out=t_[:],
                                in_=dram[:].rearrange("(ko ki) -> ki ko", ki=P))
            return t_

        bq_c = load_cols(bq_d, CCH)
        bk_c = load_cols(bk_d, CCH)
        b1_c = load_cols(b1_d, HCH)

        def load_row_bcast(dram):
            # [C] -> [128, C], replicated across partitions by the DMA
            t_ = singles.tile([P, C], F32, tag=f"row_{dram.name}",
                              name=f"row_{dram.name}")
            src = dram[:]
            bcast = bass.AP(tensor=src.tensor, offset=src.offset,
                            ap=[[0, P]] + list(src.ap))
            nc.gpsimd.dma_start(out=t_[:], in_=bcast)
            return t_

        g1_row = load_row_bcast(ln1_g)
        b1_row = load_row_bcast(ln1_b)
        bo_row = load_row_bcast(bo_d)
        b2_row = load_row_bcast(b2_d)
        bv_row = load_row_bcast(bv_d)

        def load_w(pool, dram, kos, cols, eng=None):
            t_ = pool.tile([P, kos, cols], F32R, tag=f"w_{dram.name}",
                           name=f"w_{dram.name}")
            (eng or nc.sync).dma_start(
                out=t_[:], in_=dram[:, :].rearrange("(ko ki) c -> ki ko c", ki=P))
            return t_

        # ---------------- activations + LN1 ----------------
        x_nat = iop.tile([P, NCH, C], F32, tag="x_nat")

        big = tc.alloc_tile_pool(name="big", bufs=3)
        ln1p = tc.alloc_tile_pool(name="ln1p", bufs=1)
        t_nat = ln1p.tile([P, MCH, C], F32, tag="t_nat")
        xp_nat = ln1p.tile([P, NCH, C], F32, tag="xp_nat")
        posp = tc.alloc_tile_pool(name="posp", bufs=1)
        pos_nat = posp.tile([P, NCH, C], F32, tag="pos_nat")

        nc.sync.dma_start(out=x_nat[:],
                          in_=x_d[:, :].rearrange("(no p) c -> p no c", p=P))
        nc.sync.dma_start(out=pos_nat[:],
                          in_=pos_d[:, :].rearrange("(no p) c -> p no c", p=P))
        nc.sync.dma_start(out=t_nat[:],
                          in_=t_d[:, :].rearrange("(no p) c -> p no c", p=P))

        Wq = load_w(wqkv, Wq_d, CCH, C)
        Wk = load_w(wqkv, Wk_d, CCH, C)
        Wv = load_w(wqkv, Wv_d, CCH, C)
        Wo = load_w(wqkv, Wo_d, CCH, C)
        W1 = load_w(wffn1, W1_d, CCH, HID)

        xpT = big.tile([P, CCH, N], F32R, tag="big")
        ttT = big.tile([P, CCH, M], F32R, tag="big")
        pt1 = tc.alloc_tile_pool(name="pt1", bufs=4, space="PSUM")

        layer_norm_std(nc, stats, x_nat, xp_nat, NCH, eps_t)  # xp = x_hat
        # xp = x_hat * g1 + (pos + b1); adds on the otherwise-idle GpSimd
        for ch in range(NCH):
            nc.gpsimd.tensor_add(pos_nat[:, ch, :], pos_nat[:, ch, :],
                                 b1_row[:])
            nc.vector.tensor_mul(xp_nat[:, ch, :], xp_nat[:, ch, :], g1_row[:])
            nc.gpsimd.tensor_add(xp_nat[:, ch, :], xp_nat[:, ch, :],
                                 pos_nat[:, ch, :])
        transpose_to_T(pt1, xp_nat, xpT, ident, NCH, CCH)
        posp.release()

        layer_norm_std(nc, stats, t_nat, t_nat, MCH, eps_t)   # t_hat in place
        transpose_to_T(pt1, t_nat, ttT, ident, MCH, CCH)
        pt1.release()
        ln1p.release()

        # ---------------- Q/K/V projections ----------------
        qT = big.tile([P, CCH, N], F32R, tag="big")
        kT = big.tile([P, CCH, M], F32R, tag="big")

        pp_proj = tc.alloc_tile_pool(name="pp_proj", bufs=2, space="PSUM")
        for (Wt, srcT, dstT, bias_c) in ((Wq, xpT, qT, bq_c), (Wk, ttT, kT, bk_c)):
            for cc in range(CCH):
                ps2 = pp_proj.tile([P, 2, NT], F32, tag="mm")
                for j in range(N // NT):
                    for kc in range(CCH):
                        nc.tensor.matmul(
                            ps2[:, j, :], Wt[:, kc, cc * P:(cc + 1) * P],
                            r(srcT[:, kc, j * NT:(j + 1) * NT]),
                            start=(kc == 0), stop=(kc == CCH - 1))
                nc.scalar.activation(
                    r(dstT[:, cc, :]), ps2[:, :, :],
                    AF.Identity, bias=bias_c[:, cc:cc + 1], scale=1.0)

        # v natural (rows = m), augmented with a ones column per head
        vaugp = tc.alloc_tile_pool(name="vaugp", bufs=1)
        v_aug = vaugp.tile([P, MCH, H, D + 1], F32R, tag="v_aug")
        # ones column per head: real-data broadcast DMA + per-chunk DVE copies
        ones8 = singles.tile([P, H], F32R, tag="ones8")
        vo = vones_d[:]
        nc.gpsimd.dma_start(
            out=ones8[:],
            in_=bass.AP(tensor=vo.tensor, offset=vo.offset,
                        ap=[[0, P]] + list(vo.ap)))
        for mc in range(MCH):
            nc.vector.tensor_copy(v_aug[:, mc, :, D:D + 1], ones8[:])
        bvb_v = bv_row[:].rearrange("p (h d) -> p h d", d=D)
        for mc in range(MCH):
            ps2 = pp_proj.tile([P, 2, NT], F32, tag="mm")
            for kc in range(CCH):
                nc.tensor.matmul(
                    ps2[:, 0, :], r(ttT[:, kc, mc * P:(mc + 1) * P]), Wv[:, kc, :],
                    start=(kc == 0), stop=(kc == CCH - 1))
            nc.vector.tensor_add(
                r(v_aug[:, mc, :, 0:D]),
                ps2[:, 0, :].rearrange("p (h d) -> p h d", d=D), bvb_v)
        pp_proj.release()

        # ---------------- attention (j-outer) + interleaved o-proj --------
        oT = big.tile([P, CCH, N], F32R, tag="big")
        attp = tc.alloc_tile_pool(name="attp", bufs=2)
        npool = tc.alloc_tile_pool(name="npool", bufs=2)
        pp_out = tc.alloc_tile_pool(name="pp_out", bufs=2, space="PSUM")
        spool = tc.alloc_tile_pool(name="spool", bufs=2, space="PSUM")
        opool = tc.alloc_tile_pool(name="opool", bufs=2, space="PSUM")

        for ch in range(NCH):
            nc.gpsimd.tensor_add(x_nat[:, ch, :], x_nat[:, ch, :], bo_row[:])

        for j in range(N // NT):
            for hp in range(H // 2):
                # head pair (2hp, 2hp+1): lhsT base partitions 0 / 64 land in
                # disjoint PE row groups, so their K=64 scores matmuls overlap
                cc_h = hp
                exs = [attp.tile([P, MCH, NT], F32R, tag="ex", name=f"ex{k}")
                       for k in range(2)]
                for m2 in range(MCH // 2):
                    sps2 = [spool.tile([P, 2, NT], F32, tag="sps",
                                       name=f"sps{k}") for k in range(2)]
                    for k2 in range(2):
                        mc = m2 * 2 + k2
                        for k in range(2):
                            off = k * D
                            nc.tensor.matmul(
                                sps2[k][:, k2, :],
                                r(kT[off:off + D, cc_h, mc * P:(mc + 1) * P]),
                                r(qT[off:off + D, cc_h, j * NT:(j + 1) * NT]),
                                start=True, stop=True)
                    for k in range(2):
                        nc.scalar.activation(
                            r(exs[k][:, m2 * 2:m2 * 2 + 2, :]),
                            sps2[k][:, :, :], AF.Exp, bias=0.0, scale=0.125)
                for k in range(2):
                    h = 2 * hp + k
                    off = k * D
                    ops = opool.tile([D + 1, NT], F32, tag="ops")
                    for mc in range(MCH):
                        nc.tensor.matmul(
                            ops[:], r(v_aug[:, mc, h, :]), r(exs[k][:, mc, :]),
                            start=(mc == 0), stop=(mc == MCH - 1))
                    rcp = npool.tile([1, NT], F32, tag="rcp")
                    nc.vector.reciprocal(rcp[:], ops[D:D + 1, :])
                    rcpb = npool.tile([D, NT], F32, tag="rcpb")
                    nc.gpsimd.partition_broadcast(rcpb[:], rcp[:])
                    nc.vector.tensor_mul(
                        r(oT[off:off + D, cc_h, j * NT:(j + 1) * NT]),
                        ops[0:D, :], rcpb[:])
            # rows j*512..(j+1)*512 fully attended: project + residual now
            # (PE work that overlaps the next half's ACT-bound softmax)
            for n2 in range(NT // P):
                nc_i = j * (NT // P) + n2
                ps = pp_out.tile([P, C], F32, tag="mmo")
                for cc in range(CCH):
                    nc.tensor.matmul(
                        ps[:], r(oT[:, cc, nc_i * P:(nc_i + 1) * P]),
                        Wo[:, cc, :],
                        start=(cc == 0), stop=(cc == CCH - 1))
                nc.vector.tensor_add(x_nat[:, nc_i, :], ps[:], x_nat[:, nc_i, :])

        opool.release()
        spool.release()
        npool.release()
        attp.release()
        vaugp.release()
        big.release()

        # ---------------- LN2 + transpose ----------------
        pp_ffn = tc.alloc_tile_pool(name="pp_ffn", bufs=4, space="PSUM")
        x2tp = tc.alloc_tile_pool(name="x2tp", bufs=1)
        x2T = x2tp.tile([P, CCH, N], F32R, tag="x2T")
        ln2p = tc.alloc_tile_pool(name="ln2p", bufs=1)
        xh2 = ln2p.tile([P, NCH, C], F32, tag="xh2")
        layer_norm_std(nc, stats, x_nat, xh2, NCH, eps_t)
        pt2 = tc.alloc_tile_pool(name="pt2", bufs=2, space="PSUM")
        transpose_to_T(pt2, xh2, x2T, ident, NCH, CCH)
        pt2.release()
        ln2p.release()

        wffn2 = tc.alloc_tile_pool(name="wffn2", bufs=1)
        W2 = load_w(wffn2, W2_d, HCH, C, eng=nc.gpsimd)

        # ---------------- FFN (in-place residual over x_nat) ----------------
        for ch in range(NCH):
            nc.vector.tensor_add(x_nat[:, ch, :], x_nat[:, ch, :], b2_row[:])

        ffnp = tc.alloc_tile_pool(name="ffnp", bufs=1)
        for j in range(N // NT):
            hT = ffnp.tile([P, HCH, NT], F32R, tag="hT")
            hb = ffnp.tile([P, HCH, NT], F32, tag="hb")
            for hc in range(HCH):
                ps = pp_ffn.tile([P, NT], F32, tag="mm")
                for kc in range(CCH):
                    nc.tensor.matmul(
                        ps[:], W1[:, kc, hc * P:(hc + 1) * P],
                        r(x2T[:, kc, j * NT:(j + 1) * NT]),
                        start=(kc == 0), stop=(kc == CCH - 1))
                # z = b1' + ps ; lrelu via (z * 0.01) max z on the DVE
                nc.scalar.activation(hb[:, hc, :], ps[:], AF.Identity,
                                     bias=b1_c[:, hc:hc + 1], scale=1.0)
                nc.vector.scalar_tensor_tensor(
                    out=r(hT[:, hc, :]), in0=hb[:, hc, :], scalar=0.01,
                    in1=hb[:, hc, :], op0=ALU.mult, op1=ALU.max)
            for n2 in range(NT // P):
                nc_i = j * (NT // P) + n2
                ps = pp_ffn.tile([P, C], F32, tag="mm")
                for hc in range(HCH):
                    nc.tensor.matmul(
                        ps[:], r(hT[:, hc, n2 * P:(n2 + 1) * P]), W2[:, hc, :],
                        start=(hc == 0), stop=(hc == HCH - 1))
                nc.vector.tensor_add(x_nat[:, nc_i, :], ps[:], x_nat[:, nc_i, :])
                nc.sync.dma_start(
                    out=out_d[:, :].rearrange("(no p) c -> p no c", p=P)[:, nc_i, :],
                    in_=x_nat[:, nc_i, :])

        ffnp.release()
        wffn2.release()
        pp_ffn.release()
        x2tp.release()
        pp_out.release()
        stats.release()
        iop.release()
        wffn1.release()
        wqkv.release()
        singles.release()

    nc.compile()
    return nc


_NC = None


def _get_nc():
    global _NC
    if _NC is None:
        _NC = build_nc()
    return _NC


def _prep_host(inputs):
    """Fold LN affine params into the K/V/FFN1 weights (host-side, O(C^2))."""
    def f(k):
        return np.asarray(inputs[k], np.float32)

    g1, b1g = f("ln1_g"), f("ln1_b")
    g2, b2g = f("ln2_g"), f("ln2_b")
    Wk, Wv, W1 = f("Wk"), f("Wv"), f("W1")
    out = {
        "ln1_g": g1, "ln1_b": b1g,
        "Wq": f("Wq"), "bq": f("bq"),
        "Wk": g1[:, None] * Wk, "bk": b1g @ Wk + f("bk"),
        "Wv": g1[:, None] * Wv, "bv": b1g @ Wv + f("bv"),
        "Wo": f("Wo"), "bo": f("bo"),
        "W1": g2[:, None] * W1, "b1": b2g @ W1 + f("b1"),
        "W2": f("W2"), "b2": f("b2"),
        "vones": np.ones([8], np.float32),
    }
    return {k: np.ascontiguousarray(v, np.float32) for k, v in out.items()}


def run(inputs, trace=False):
    from concourse.bass_utils import run_bass_kernel_spmd

    nc = _get_nc()
    shared = _prep_host(inputs)
    in_maps = []
    for b in range(B):
        m = dict(shared)
        m["x"] = np.ascontiguousarray(np.asarray(inputs["x"][b], np.float32))
        m["t"] = np.ascontiguousarray(np.asarray(inputs["t"][b], np.float32))
        m["pos"] = np.ascontiguousarray(np.asarray(inputs["pos"][b], np.float32))
        in_maps.append(m)
    res = run_bass_kernel_spmd(nc, in_maps, core_ids=list(range(B)), trace=trace)
    out = np.stack([res.results[b]["out"] for b in range(B)], axis=0)
    return out, res


def kernel(**inputs):
    out, _ = run(inputs, trace=False)
    return out

